# revision 23
# baseline (speedup 1.0000x reference)
"""CenterLoss kernel for Trainium2 (8 NeuronCores, Bass).

Math: the reference builds the full [B, C] squared-distance matrix, masks it
to one column per row (the label), clips ALL entries to [1e-12, 1e12], sums
and divides by B.  Because the mask keeps exactly one entry per row:

    loss = ( sum_b clip(||x_b - centers[l_b]||^2, 1e-12, 1e12)
             + (B*C - B) * 1e-12 ) / B

so the kernel is a row gather of `centers` plus an elementwise reduction --
no GEMM needed.  The per-row sums are ~chi^2(2048) (mean ~4096), so the
clip bounds can never bind on the gathered entries; they are dropped on
device and the (B*C - B)*1e-12 constant is added on host.

Sharding: data-parallel over the batch.  Each of the 8 cores receives 512
rows of x (host-cast to bf16 and pre-wrapped to the SBUF tile layout for
8 KB DMA descriptors), their labels ([128, 4] int32), and the full centers
table host-cast to fp8 e3m4 (4 mantissa bits; squared-distance bias
~1e-4, far under the 2e-2 gate).  The kernel is DMA-throughput-bound
(~23 GB/s per SDMA engine), so both casts cut wall time directly.

On device: x loads on the sync HWDGE ring, labels on the scalar HWDGE
ring; center rows are gathered with SWDGE indirect DMA, casting
fp8 -> bf16 in-flight (tiles 0-2 full rows, tile 3 in column halves so
the tail compute chain is short).  Compute is split across DVE and ACT:
a custom fused DVE op (out = (x-c)^2, accum_out = f32 row-sum) handles
tiles 0, 2 and 3b in one pass each, while ACT squares tiles 1 and 3a
(DVE does the bf16 2x-rate subtracts) with fused f32 row-sum accumulate.
A ones-matmul partition reduction -> DVE reduce -> scalar register store
produces the per-core scalar; host adds the 8 partial sums plus the clip
constant.
"""

import numpy as np
import ml_dtypes
from contextlib import ExitStack
from operator import add as _operator_add

import concourse.bacc as bacc
import concourse.bass as bass
import concourse.mybir as mybir
import concourse.dve_ops as dve_ops_mod
from concourse.dve_spec import Spec, Src0, Src1, Zero, sq, lower, _has_src1
from concourse.dve_uop import DveOpSpec
from concourse.bass_utils import run_bass_kernel_spmd

B = 4096
D = 2048
C = 8192
N_CORES = 8
SHARD = B // N_CORES          # 512
P = 128
T = SHARD // P                # 4
DH = D // 2                   # 1024 (column halves of the last tile)

_nc_cache = None


def _register_sqdiff():
    """Register a fused (x-c)^2 row-sum op via the documented custom-DVE
    extension API (dve_ops is append-only at runtime; the repo checkout is
    read-only).  body runs per element; accum folds the row sum into a
    [P, 1] f32 output."""
    name = "SQDIFF_SUM_ANT"
    for op in dve_ops_mod.OPS:
        if op.name == name:
            return op

    def _ref(in0, in1, *a):
        b = (in0.astype(np.float32) - in1.astype(np.float32)) ** 2
        return b, b.reshape(b.shape[0], -1).sum(axis=-1, keepdims=True)

    spec = Spec(body=sq(Src0 - Src1), accum=_operator_add, accum_init=Zero,
                reference=_ref)
    row = max(dve_ops_mod._SUB_OPCODE_FOR_NAME.values()) + 1
    assert row < 0x20
    dve_ops_mod._SUB_OPCODE_FOR_NAME[name] = row
    shas = {}
    for ver in ("v3", "v4"):
        uops = lower(spec, ver=ver)
        shas[ver] = DveOpSpec(
            name=name, opcode=row, uops=uops, rd1_en=_has_src1(spec)
        ).sha(ver)
    op = dve_ops_mod.DveOp(name, spec, subdim=False, uops_sha=shas)
    dve_ops_mod.OPS.append(op)
    dve_ops_mod.CUSTOM_DVE_SPECS[name] = spec
    return op


_SQDIFF = _register_sqdiff()


def _build():
    global _nc_cache
    if _nc_cache is not None:
        return _nc_cache

    nc = bacc.Bacc("TRN2", target_bir_lowering=False, debug=False,
                   dynamic_dma_scratch_size=16384)
    bf16 = mybir.dt.bfloat16
    f32 = mybir.dt.float32
    # x is host-wrapped to the SBUF tile layout: xw[p, t*D:(t+1)*D] =
    # x[t*128+p, :], so one DMA covering tiles (t, t+1) moves 8 KB
    # contiguous per partition (best HWDGE descriptor size).
    x = nc.dram_tensor("x", [P, T * D], bf16, kind="ExternalInput")
    labels = nc.dram_tensor("labels", [P, T], mybir.dt.int32, kind="ExternalInput")
    centers = nc.dram_tensor("centers", [C, D], mybir.dt.float8e3, kind="ExternalInput")
    out = nc.dram_tensor("out", [1, 1], f32, kind="ExternalOutput")

    with ExitStack() as ctx:
        block = ctx.enter_context(nc.Block(no_gpsimd_drain=True))
        lab = ctx.enter_context(nc.sbuf_tensor("lab", [P, T], mybir.dt.int32))
        xw = ctx.enter_context(nc.sbuf_tensor("xw", [P, T * D], bf16))
        gts = [ctx.enter_context(nc.sbuf_tensor(f"gt{t}", [P, D], bf16)) for t in range(T)]
        # subtract targets for the ACT-pipeline tiles (1, 2, 3a)
        ds1 = ctx.enter_context(nc.sbuf_tensor("ds1", [P, D], bf16))
        ds2 = ctx.enter_context(nc.sbuf_tensor("ds2", [P, D], bf16))
        ds3 = ctx.enter_context(nc.sbuf_tensor("ds3", [P, DH], bf16))
        # elementwise-output dump for the fused DVE op (value unused)
        dump = ctx.enter_context(nc.sbuf_tensor("dump", [P, D], bf16))
        # rowsum[:, k]: k=0 tile0 (DVE), 1/2 tiles 1-2 (ACT), 3 tile3a (ACT),
        # 4 tile3b (DVE)
        rowsum = ctx.enter_context(nc.sbuf_tensor("rowsum", [P, T + 1], f32))
        ones = ctx.enter_context(nc.sbuf_tensor("ones", [P, 1], f32))
        final = ctx.enter_context(nc.sbuf_tensor("final", [1, 1], f32))
        colsum = ctx.enter_context(nc.psum_tensor("colsum", [1, T + 1], f32))

        s_lab = ctx.enter_context(nc.semaphore("s_lab"))
        s_xa = ctx.enter_context(nc.semaphore("s_xa"))   # tiles 0-1
        s_xb = ctx.enter_context(nc.semaphore("s_xb"))   # tiles 2-3
        s_g = [ctx.enter_context(nc.semaphore(f"s_g{k}")) for k in range(5)]
        s_sub = ctx.enter_context(nc.semaphore("s_sub"))
        s_acc = ctx.enter_context(nc.semaphore("s_acc"))
        s_mm = ctx.enter_context(nc.semaphore("s_mm"))
        s_red = ctx.enter_context(nc.semaphore("s_red"))

        @block.sync
        def _(sync):
            sync.dma_start(
                out=xw[:, 0:2 * D], in_=x[:, 0:2 * D]
            ).then_inc(s_xa, 16)
            sync.dma_start(
                out=xw[:, 2 * D:4 * D], in_=x[:, 2 * D:4 * D]
            ).then_inc(s_xb, 16)

        @block.scalar
        def _(scalar):
            scalar.dma_start(out=lab[:, :], in_=labels[:, :]).then_inc(s_lab, 16)
            for i, (src, col) in enumerate(
                ((ds1, 0), (ds2, 2))
            ):
                scalar.wait_ge(s_sub, i + 1)
                scalar.activation(
                    out=src[:, :], in_=src[:, :],
                    func=mybir.ActivationFunctionType.Square,
                    accum_out=rowsum[:, col:col + 1],
                ).then_inc(s_acc, 1)
            with scalar.register("gr_out") as gr_out:
                scalar.wait_ge(s_red, 1)
                scalar.reg_load(gr_out, final[:1, :1].bitcast(mybir.dt.int32))
                scalar.reg_save(out[:1, :1].bitcast(mybir.dt.int32), gr_out)

        @block.gpsimd
        def _(gpsimd):
            gpsimd.wait_ge(s_lab, 16)
            for t in range(3):
                gpsimd.indirect_dma_start(
                    out=gts[t][:, :],
                    out_offset=None,
                    in_=centers[:, :],
                    in_offset=bass.IndirectOffsetOnAxis(ap=lab[:, t:t + 1], axis=0),
                ).then_inc(s_g[t], 16)
            for h in range(2):
                gpsimd.indirect_dma_start(
                    out=gts[3][:, h * DH:(h + 1) * DH],
                    out_offset=None,
                    in_=centers[:, :],
                    in_offset=bass.IndirectOffsetOnAxis(ap=lab[:, 3:4], axis=0),
                    element_offset=h * DH,
                ).then_inc(s_g[3 + h], 16)

        @block.vector
        def _(vector):
            vector.memset(ones[:, :], 1.0)
            # tile 0: quick subtract first so ACT starts on the first arrival
            vector.wait_ge(s_xa, 16)
            vector.wait_ge(s_g[0], 16)
            vector.tensor_tensor(
                out=ds1[:, :], in0=xw[:, 0:D], in1=gts[0][:, :],
                op=mybir.AluOpType.subtract,
            ).then_inc(s_sub, 1)
            # tile 1: fused sqdiff+rowsum on DVE
            vector.wait_ge(s_g[1], 16)
            vector._custom_dve(
                _SQDIFF, out=dump[:, :], in0=xw[:, D:2 * D], in1=gts[1][:, :],
                accum_out=rowsum[:, 1:2],
            ).then_inc(s_acc, 1)
            # tile 2: subtract for ACT
            vector.wait_ge(s_xb, 16)
            vector.wait_ge(s_g[2], 16)
            vector.tensor_tensor(
                out=ds2[:, :], in0=xw[:, 2 * D:3 * D], in1=gts[2][:, :],
                op=mybir.AluOpType.subtract,
            ).then_inc(s_sub, 1)
            # tile 3a: fused on DVE
            vector.wait_ge(s_g[3], 16)
            vector._custom_dve(
                _SQDIFF, out=dump[:, :DH], in0=xw[:, 3 * D:3 * D + DH],
                in1=gts[3][:, :DH],
                accum_out=rowsum[:, 3:4],
            ).then_inc(s_acc, 1)
            # tile 3b: fused on DVE (short tail)
            vector.wait_ge(s_g[4], 16)
            vector._custom_dve(
                _SQDIFF, out=dump[:, DH:], in0=xw[:, 3 * D + DH:4 * D],
                in1=gts[3][:, DH:],
                accum_out=rowsum[:, 4:5],
            ).then_inc(s_acc, 1)
            vector.wait_ge(s_mm, 1)
            vector.tensor_reduce(
                out=final[:, :], in_=colsum[:1, :],
                axis=mybir.AxisListType.X, op=mybir.AluOpType.add,
            ).then_inc(s_red, 1)

        @block.tensor
        def _(tensor):
            tensor.wait_ge(s_acc, 5)
            tensor.matmul(
                colsum[:1, :], ones[:, :], rowsum[:, :], start=True, stop=True
            ).then_inc(s_mm, 1)

    nc.compile()
    _nc_cache = nc
    return nc


def _to_bf16(a):
    return np.ascontiguousarray(np.asarray(a, dtype=np.float32)).astype(
        ml_dtypes.bfloat16
    )


def _make_in_maps(x, labels, centers):
    x16 = _to_bf16(x)
    c8 = np.ascontiguousarray(np.asarray(centers, dtype=np.float32)).astype(
        ml_dtypes.float8_e3m4
    )
    lab32 = np.asarray(labels).astype(np.int32)
    in_maps = []
    for i in range(N_CORES):
        sl = slice(i * SHARD, (i + 1) * SHARD)
        lab_w = np.ascontiguousarray(lab32[sl].reshape(T, P).T)
        # wrap x to SBUF tile layout: xw[p, t*D:(t+1)*D] = x[sl][t*128+p]
        x_w = np.ascontiguousarray(
            x16[sl].reshape(T, P, D).transpose(1, 0, 2).reshape(P, T * D)
        )
        in_maps.append({
            "x": x_w,
            "labels": lab_w,
            "centers": c8,
        })
    return in_maps


def _aggregate(results):
    total = sum(float(r["out"][0, 0]) for r in results)
    total += (B * C - B) * 1e-12
    return np.float32(total / B)


def kernel(x, labels, centers):
    nc = _build()
    in_maps = _make_in_maps(x, labels, centers)
    res = run_bass_kernel_spmd(nc, in_maps, core_ids=list(range(N_CORES)))
    return _aggregate(res.results)


# revision 25
# speedup vs baseline: 1.0508x; 1.0508x over previous
"""CenterLoss kernel for Trainium2 (8 NeuronCores, Bass).

Math: the reference builds the full [B, C] squared-distance matrix, masks it
to one column per row (the label), clips ALL entries to [1e-12, 1e12], sums
and divides by B.  Because the mask keeps exactly one entry per row:

    loss = ( sum_b clip(||x_b - centers[l_b]||^2, 1e-12, 1e12)
             + (B*C - B) * 1e-12 ) / B

so the kernel is a row gather of `centers` plus an elementwise reduction --
no GEMM needed.  The per-row sums are ~chi^2(2048) (mean ~4096), so the
clip bounds can never bind on the gathered entries; they are dropped on
device and the (B*C - B)*1e-12 constant is added on host.

Sharding: data-parallel over the batch.  Each of the 8 cores receives 512
rows of x (host-cast to bf16 and pre-wrapped to the SBUF tile layout for
8 KB DMA descriptors), their labels ([128, 4] int32), and the full centers
table host-cast to fp8 e3m4 (4 mantissa bits; squared-distance bias
~1e-4, far under the 2e-2 gate).  The kernel is DMA-throughput-bound
(~23 GB/s per SDMA engine), so both casts cut wall time directly.

On device: x loads on the sync HWDGE ring, labels on the scalar HWDGE
ring; center rows are gathered with SWDGE indirect DMA, casting
fp8 -> bf16 in-flight (tiles 0-2 full rows, tile 3 in column halves so
the tail compute chain is short).  Compute is split across DVE and ACT:
a custom fused DVE op (out = (x-c)^2, accum_out = f32 row-sum) handles
tiles 0, 2 and 3b in one pass each, while ACT squares tiles 1 and 3a
(DVE does the bf16 2x-rate subtracts) with fused f32 row-sum accumulate.
A ones-matmul partition reduction -> DVE reduce -> scalar register store
produces the per-core scalar; host adds the 8 partial sums plus the clip
constant.
"""

import numpy as np
import ml_dtypes
from contextlib import ExitStack
from operator import add as _operator_add

import concourse.bacc as bacc
import concourse.bass as bass
import concourse.mybir as mybir
import concourse.dve_ops as dve_ops_mod
from concourse.dve_spec import Spec, Src0, Src1, Zero, sq, lower, _has_src1
from concourse.dve_uop import DveOpSpec
from concourse.bass_utils import run_bass_kernel_spmd

B = 4096
D = 2048
C = 8192
N_CORES = 8
SHARD = B // N_CORES          # 512
P = 128
T = SHARD // P                # 4
DH = D // 2                   # 1024 (column halves of the last tile)

_nc_cache = None


def _register_sqdiff():
    """Register a fused (x-c)^2 row-sum op via the documented custom-DVE
    extension API (dve_ops is append-only at runtime; the repo checkout is
    read-only).  body runs per element; accum folds the row sum into a
    [P, 1] f32 output."""
    name = "SQDIFF_SUM_ANT"
    for op in dve_ops_mod.OPS:
        if op.name == name:
            return op

    def _ref(in0, in1, *a):
        b = (in0.astype(np.float32) - in1.astype(np.float32)) ** 2
        return b, b.reshape(b.shape[0], -1).sum(axis=-1, keepdims=True)

    spec = Spec(body=sq(Src0 - Src1), accum=_operator_add, accum_init=Zero,
                reference=_ref)
    row = max(dve_ops_mod._SUB_OPCODE_FOR_NAME.values()) + 1
    assert row < 0x20
    dve_ops_mod._SUB_OPCODE_FOR_NAME[name] = row
    shas = {}
    for ver in ("v3", "v4"):
        uops = lower(spec, ver=ver)
        shas[ver] = DveOpSpec(
            name=name, opcode=row, uops=uops, rd1_en=_has_src1(spec)
        ).sha(ver)
    op = dve_ops_mod.DveOp(name, spec, subdim=False, uops_sha=shas)
    dve_ops_mod.OPS.append(op)
    dve_ops_mod.CUSTOM_DVE_SPECS[name] = spec
    return op


_SQDIFF = _register_sqdiff()


def _build():
    global _nc_cache
    if _nc_cache is not None:
        return _nc_cache

    nc = bacc.Bacc("TRN2", target_bir_lowering=False, debug=False,
                   dynamic_dma_scratch_size=16384)
    bf16 = mybir.dt.bfloat16
    f32 = mybir.dt.float32
    # x is host-wrapped to the SBUF tile layout: xw[p, t*D:(t+1)*D] =
    # x[t*128+p, :], so one DMA covering tiles (t, t+1) moves 8 KB
    # contiguous per partition (best HWDGE descriptor size).
    x = nc.dram_tensor("x", [P, T * D], bf16, kind="ExternalInput")
    labels = nc.dram_tensor("labels", [P, T], mybir.dt.int32, kind="ExternalInput")
    centers = nc.dram_tensor("centers", [C, D], mybir.dt.float8e3, kind="ExternalInput")
    out = nc.dram_tensor("out", [1, 1], f32, kind="ExternalOutput")

    with ExitStack() as ctx:
        block = ctx.enter_context(nc.Block(no_gpsimd_drain=True))
        lab = ctx.enter_context(nc.sbuf_tensor("lab", [P, T], mybir.dt.int32))
        xw = ctx.enter_context(nc.sbuf_tensor("xw", [P, T * D], bf16))
        gts = [ctx.enter_context(nc.sbuf_tensor(f"gt{t}", [P, D], bf16)) for t in range(T)]
        # subtract targets for the ACT-pipeline tiles (1, 2, 3a)
        ds1 = ctx.enter_context(nc.sbuf_tensor("ds1", [P, D], bf16))
        ds2 = ctx.enter_context(nc.sbuf_tensor("ds2", [P, D], bf16))
        ds3 = ctx.enter_context(nc.sbuf_tensor("ds3", [P, DH], bf16))
        # elementwise-output dump for the fused DVE op (value unused)
        dump = ctx.enter_context(nc.sbuf_tensor("dump", [P, D], bf16))
        # rowsum[:, k]: k=0 tile0 (DVE), 1/2 tiles 1-2 (ACT), 3 tile3a (ACT),
        # 4 tile3b (DVE)
        rowsum = ctx.enter_context(nc.sbuf_tensor("rowsum", [P, T + 1], f32))
        ones = ctx.enter_context(nc.sbuf_tensor("ones", [P, 1], f32))
        final = ctx.enter_context(nc.sbuf_tensor("final", [1, 1], f32))
        colsum = ctx.enter_context(nc.psum_tensor("colsum", [1, T + 1], f32))

        s_lab = ctx.enter_context(nc.semaphore("s_lab"))
        s_xa = ctx.enter_context(nc.semaphore("s_xa"))   # tiles 0-1
        s_xb = ctx.enter_context(nc.semaphore("s_xb"))   # tiles 2-3
        s_g = [ctx.enter_context(nc.semaphore(f"s_g{k}")) for k in range(T)]
        s_sub = ctx.enter_context(nc.semaphore("s_sub"))
        s_acc = ctx.enter_context(nc.semaphore("s_acc"))
        s_mm = ctx.enter_context(nc.semaphore("s_mm"))
        s_red = ctx.enter_context(nc.semaphore("s_red"))

        @block.sync
        def _(sync):
            sync.dma_start(
                out=xw[:, 0:2 * D], in_=x[:, 0:2 * D]
            ).then_inc(s_xa, 16)
            sync.dma_start(
                out=xw[:, 2 * D:4 * D], in_=x[:, 2 * D:4 * D]
            ).then_inc(s_xb, 16)

        @block.scalar
        def _(scalar):
            scalar.dma_start(out=lab[:, :], in_=labels[:, :]).then_inc(s_lab, 16)
            for i, (src, col) in enumerate(
                ((ds1, 1), (ds3, 3))
            ):
                scalar.wait_ge(s_sub, i + 1)
                scalar.activation(
                    out=src[:, :], in_=src[:, :],
                    func=mybir.ActivationFunctionType.Square,
                    accum_out=rowsum[:, col:col + 1],
                ).then_inc(s_acc, 1)
            with scalar.register("gr_out") as gr_out:
                scalar.wait_ge(s_red, 1)
                scalar.reg_load(gr_out, final[:1, :1].bitcast(mybir.dt.int32))
                scalar.reg_save(out[:1, :1].bitcast(mybir.dt.int32), gr_out)

        @block.gpsimd
        def _(gpsimd):
            gpsimd.wait_ge(s_lab, 16)
            for t in range(T):
                gpsimd.indirect_dma_start(
                    out=gts[t][:, :],
                    out_offset=None,
                    in_=centers[:, :],
                    in_offset=bass.IndirectOffsetOnAxis(ap=lab[:, t:t + 1], axis=0),
                ).then_inc(s_g[t], 16)

        @block.vector
        def _(vector):
            vector.memset(ones[:, :], 1.0)
            # tile 0: fused sqdiff+rowsum on DVE
            vector.wait_ge(s_xa, 16)
            vector.wait_ge(s_g[0], 16)
            vector._custom_dve(
                _SQDIFF, out=dump[:, :], in0=xw[:, 0:D], in1=gts[0][:, :],
                accum_out=rowsum[:, 0:1],
            ).then_inc(s_acc, 1)
            # tile 1: bf16 2x subtract; ACT squares it
            vector.wait_ge(s_g[1], 16)
            vector.tensor_tensor(
                out=ds1[:, :], in0=xw[:, D:2 * D], in1=gts[1][:, :],
                op=mybir.AluOpType.subtract,
            ).then_inc(s_sub, 1)
            # tile 2: fused on DVE
            vector.wait_ge(s_xb, 16)
            vector.wait_ge(s_g[2], 16)
            vector._custom_dve(
                _SQDIFF, out=dump[:, :], in0=xw[:, 2 * D:3 * D], in1=gts[2][:, :],
                accum_out=rowsum[:, 2:3],
            ).then_inc(s_acc, 1)
            # tile 3a: subtract for ACT
            vector.wait_ge(s_g[3], 16)
            vector.tensor_tensor(
                out=ds3[:, :], in0=xw[:, 3 * D:3 * D + DH], in1=gts[3][:, :DH],
                op=mybir.AluOpType.subtract,
            ).then_inc(s_sub, 1)
            # tile 3b: fused sqdiff+rowsum on DVE (short tail)
            vector._custom_dve(
                _SQDIFF, out=dump[:, :DH], in0=xw[:, 3 * D + DH:4 * D],
                in1=gts[3][:, DH:],
                accum_out=rowsum[:, 4:5],
            ).then_inc(s_acc, 1)
            vector.wait_ge(s_mm, 1)
            vector.tensor_reduce(
                out=final[:, :], in_=colsum[:1, :],
                axis=mybir.AxisListType.X, op=mybir.AluOpType.add,
            ).then_inc(s_red, 1)

        @block.tensor
        def _(tensor):
            tensor.wait_ge(s_acc, 5)
            tensor.matmul(
                colsum[:1, :], ones[:, :], rowsum[:, :], start=True, stop=True
            ).then_inc(s_mm, 1)

    nc.compile()
    _nc_cache = nc
    return nc


def _to_bf16(a):
    return np.ascontiguousarray(np.asarray(a, dtype=np.float32)).astype(
        ml_dtypes.bfloat16
    )


def _make_in_maps(x, labels, centers):
    x16 = _to_bf16(x)
    c8 = np.ascontiguousarray(np.asarray(centers, dtype=np.float32)).astype(
        ml_dtypes.float8_e3m4
    )
    lab32 = np.asarray(labels).astype(np.int32)
    in_maps = []
    for i in range(N_CORES):
        sl = slice(i * SHARD, (i + 1) * SHARD)
        lab_w = np.ascontiguousarray(lab32[sl].reshape(T, P).T)
        # wrap x to SBUF tile layout: xw[p, t*D:(t+1)*D] = x[sl][t*128+p]
        x_w = np.ascontiguousarray(
            x16[sl].reshape(T, P, D).transpose(1, 0, 2).reshape(P, T * D)
        )
        in_maps.append({
            "x": x_w,
            "labels": lab_w,
            "centers": c8,
        })
    return in_maps


def _aggregate(results):
    total = sum(float(r["out"][0, 0]) for r in results)
    total += (B * C - B) * 1e-12
    return np.float32(total / B)


def kernel(x, labels, centers):
    nc = _build()
    in_maps = _make_in_maps(x, labels, centers)
    res = run_bass_kernel_spmd(nc, in_maps, core_ids=list(range(N_CORES)))
    return _aggregate(res.results)
